# revision 35
# baseline (speedup 1.0000x reference)
"""Trainium2 Bass kernel for gated short-time-warp + Conv1d (nn_GW_Conv1D).

Reference computation (per batch element b, C=64 channels, T=32768):
  g = tanh(einsum('ct,c->t', x, est_w)) * 0.5            # velocity, |g| <= 0.5
  d = flow(g)    per 256-window (scaling & squaring, 4 iters), |d| <= 0.5
  xw = interp1d(x, p + d)   per window                    # forward warp
  y = conv1d(xw, conv_w, conv_b, k=3, SAME)               # channel mixing
  d_inv = flow(-g); out = interp1d(y, p + d_inv)          # inverse warp

|d| < 1 always, so each warp is a 3-term elementwise stencil:
  out = x*(1-dn-dp) + x[-1]*dn + x[+1]*dp,  dn=relu(-d), dp=relu(d)
with dn zeroed at window starts and dp zeroed at window ends (the clip).
The edge zeroing is folded into d itself before broadcasting:
  d[win col 0] <- max(d, 0)   (kills dn only; dp unchanged)
  d[win col W-1] <- min(d, 0) (kills dp only; dn unchanged)

v10 architecture (single streaming pass, fp16 datapath):
  - "halves" layout everywhere: partition p = (h, c), free dim = t in half.
  - x streamed fp32 in two phases through the xw tile's memory (bitcast
    staging), h0 on the sync ring / h1 on the scalar ring (disjoint
    even/odd SDMA engine sets); the fp32 einsum runs straight off the
    staging while the DVE casts it to the persistent fp16 x tile.
  - flow on [128 windows, 512] fp16 (fwd | inv), all-DVE.
  - per-window warp coefficients: derive dn/dp small, flatten to 8-replica
    (dn,dp)-interleaved rows, then per chunk an 8-DMA depth-4 doubling
    tree broadcasts to [128, 2*CH] (fwd split over sync+scalar rings,
    inv on the gpsimd SWDGE ring; stores on sync).
  - 6-op warp entirely on DVE fp16 2x: s1=(x[-1]-x)*dn; s2=(x[+1]-x)*dp;
    out = x+s1+s2.
  - conv as block-diagonal K=128 fp16 matmuls, bias fused in the
    scalar-engine PSUM evacuation.
  - output stored fp16 (host casts to fp32).

Sharding: pure data parallelism, batch b -> core b (8 cores).
"""
import sys

sys.path.insert(0, "/opt/trn_rl_repo")

import numpy as np
from contextlib import ExitStack

import concourse.bass as bass
import concourse.tile as tile
from concourse import bacc, mybir
from concourse.bass_interp import get_hw_module
from concourse import bass_utils

F32 = mybir.dt.float32
F16 = mybir.dt.float16
AF = mybir.ActivationFunctionType
ALU = mybir.AluOpType

NCORES = 8
C, T, W = 64, 32768, 256
H = T // 2            # 16384 cols per half
FLOW_ITERS = 4
CH = 4096             # main-loop chunk (window-aligned)
NCH = H // CH         # 4 chunks


def _flow(nc, pool, d2):
    """Scaling-and-squaring on d2 (128, 512) fp16 = [d_fwd | d_inv], all-DVE."""
    for _ in range(FLOW_ITERS):
        dn = pool.tile([128, 512], F16, tag="fl_dn")
        dp = pool.tile([128, 512], F16, tag="fl_dp")
        nc.vector.tensor_scalar(dn[:], d2[:], -1.0, 0.0, ALU.mult, ALU.max)
        nc.vector.tensor_scalar_max(dp[:], d2[:], 0.0)
        nc.vector.tensor_scalar_mul(dn[:, 0:1], dn[:, 0:1], 0.0)
        nc.vector.tensor_scalar_mul(dn[:, 256:257], dn[:, 256:257], 0.0)
        nc.vector.tensor_scalar_mul(dp[:, 255:256], dp[:, 255:256], 0.0)
        nc.vector.tensor_scalar_mul(dp[:, 511:512], dp[:, 511:512], 0.0)
        am = pool.tile([128, 512], F16, tag="fl_am")
        nc.vector.tensor_tensor(am[:], dn[:], dp[:], ALU.add)
        nc.vector.tensor_scalar(am[:], am[:], -1.0, 1.0, ALU.mult, ALU.add)
        itp = pool.tile([128, 512], F16, tag="fl_it")
        tmp = pool.tile([128, 512], F16, tag="fl_tm")
        nc.vector.tensor_tensor(itp[:], d2[:], am[:], ALU.mult)
        nc.vector.tensor_tensor(tmp[:, 1:512], d2[:, 0:511], dn[:, 1:512], ALU.mult)
        nc.vector.tensor_tensor(itp[:, 1:512], itp[:, 1:512], tmp[:, 1:512], ALU.add)
        nc.vector.tensor_tensor(tmp[:, 0:511], d2[:, 1:512], dp[:, 0:511], ALU.mult)
        nc.vector.tensor_tensor(itp[:, 0:511], itp[:, 0:511], tmp[:, 0:511], ALU.add)
        nc.vector.tensor_tensor(d2[:], d2[:], itp[:], ALU.add)
    # fold the window-edge clip into d itself
    for c0 in (0, 256):
        nc.vector.tensor_scalar_max(d2[:, c0:c0 + 1], d2[:, c0:c0 + 1], 0.0)
    for c0 in (255, 511):
        nc.vector.tensor_scalar_min(d2[:, c0:c0 + 1], d2[:, c0:c0 + 1], 0.0)


def _build_module():
    nc = bacc.Bacc("TRN2", target_bir_lowering=False, debug=False,
                   enable_asserts=False, num_devices=NCORES)
    x = nc.dram_tensor("x", (C, T), F32, kind="ExternalInput").ap()
    ew = nc.dram_tensor("ew", (128, 2), F32, kind="ExternalInput").ap()
    cw = nc.dram_tensor("cw", (128, 384), F16, kind="ExternalInput").ap()
    cb = nc.dram_tensor("cb", (128, 1), F32, kind="ExternalInput").ap()
    y = nc.dram_tensor("y", (C, T), F16, kind="ExternalOutput").ap()

    # per-half views: [64, H] slices of the (C, T) tensors (outer dim 64
    # keeps DMA descriptors striped across many SDMA engines)
    x_h = [x[:, 0:H], x[:, H:T]]
    y_h = [y[:, 0:H], y[:, H:T]]

    with tile.TileContext(nc) as tc, ExitStack() as ctx:
        big = ctx.enter_context(tc.tile_pool(name="big", bufs=1))
        sm = ctx.enter_context(tc.tile_pool(name="sm", bufs=1))

        # persistent tiles: x (halo'd), warped x (halo'd), flat coef rows
        x16 = big.tile([128, H + 2], F16)
        xw = big.tile([128, H + 2], F16)
        # flat coef rows: r = dir*64 + (h*2+hh)*16 + rep*2 + ci
        # (8 replicated (dn,dp) row pairs per (dir,h,hh)), cols = H/2 span hh
        flat = big.tile([128, H // 2], F16)

        ew_sb = sm.tile([128, 2], F32, tag="ew")
        nc.sync.dma_start(ew_sb[:], ew)
        cw_sb = sm.tile([128, 384], F16, tag="cw")
        nc.sync.dma_start(cw_sb[:], cw)
        cb_sb = sm.tile([128, 1], F32, tag="cb")
        nc.sync.dma_start(cb_sb[:], cb)

        # ------- Stage A: 2-phase fp32 load into xw-as-f32 staging ----------
        # einsum runs fp32 off the staging; DVE casts staging -> x16 fp16
        xwf = xw[:].bitcast(F32)                         # [128, 8193] view
        g_w = sm.tile([128, 256], F16, tag="gw")         # windows on partitions
        stageA = ctx.enter_context(tc.tile_pool(name="stA", bufs=2))
        rings = {0: nc.sync, 1: nc.scalar}
        with tc.tile_pool(name="psA", bufs=2, space="PSUM") as psA:
            for i0 in (0, H // 2):
                for h in (0, 1):
                    rings[h].dma_start(xwf[h * 64:(h + 1) * 64, 0:H // 2],
                                       x_h[h][:, i0:i0 + H // 2])
                for i in range(0, H // 2, 2048):
                    pg = psA.tile([2, 2048], F32, tag="pg")
                    for j in range(4):
                        nc.tensor.matmul(pg[:, j * 512:(j + 1) * 512], ew_sb[:],
                                         xwf[:, i + j * 512:i + (j + 1) * 512],
                                         start=True, stop=True)
                    ge = stageA.tile([2, 2048], F16, tag="ge")
                    nc.scalar.copy(ge[:], pg[:])
                    w0 = (i0 + i) // 256
                    for h in (0, 1):
                        nc.sync.dma_start(g_w[h * 64 + w0:h * 64 + w0 + 8, :],
                                          ge[h:h + 1, :])
                # cast staging to fp16 (also releases staging for phase 2)
                for h in (0, 1):
                    nc.vector.tensor_scalar_add(
                        x16[h * 64:(h + 1) * 64, 1 + i0:1 + i0 + H // 2],
                        xwf[h * 64:(h + 1) * 64, 0:H // 2], 0.0)

        # halos (staging no longer needed after the phase-2 casts)
        nc.vector.memset(xw[:, 0:1], 0.0)
        nc.vector.memset(xw[:, H + 1:H + 2], 0.0)
        nc.vector.memset(x16[:, 0:1], 0.0)
        nc.vector.memset(x16[:, H + 1:H + 2], 0.0)

        # ------- Stage B: tanh, flow, derive ------------------------------
        g_th = sm.tile([128, 256], F32, tag="gth")
        nc.scalar.activation(g_th[:], g_w[:], AF.Tanh)
        d2 = sm.tile([128, 512], F16, tag="d2")          # [d_fwd | d_inv]
        nc.vector.tensor_scalar_mul(d2[:, 0:256], g_th[:], 0.5 / 16.0)
        nc.vector.tensor_scalar_mul(d2[:, 256:512], g_th[:], -0.5 / 16.0)
        _flow(nc, sm, d2)
        dn_s = sm.tile([128, 512], F16, tag="dn_s")
        dp_s = sm.tile([128, 512], F16, tag="dp_s")
        nc.vector.tensor_scalar(dn_s[:], d2[:], -1.0, 0.0, ALU.mult, ALU.max)
        nc.vector.tensor_scalar_max(dp_s[:], d2[:], 0.0)

        def flatten(dir_, ring):
            for ci_, coef_s in ((0, dn_s), (1, dp_s)):
                for h in (0, 1):
                    for hh in (0, 1):
                        r = dir_ * 64 + (h * 2 + hh) * 16 + ci_
                        ring.dma_start(
                            flat[r:r + 1, :],
                            coef_s[h * 64 + hh * 32:h * 64 + (hh + 1) * 32,
                                   dir_ * 256:(dir_ + 1) * 256])
            # replicate each (dn,dp) row pair to 8 copies, wave-major
            for p in (2, 4, 8):
                for blk in range(4):
                    r0 = dir_ * 64 + blk * 16
                    ring.dma_start(flat[r0 + p:r0 + 2 * p, :],
                                   flat[r0:r0 + p, :])

        # ------- main streaming loop ----------------------------------------
        cf_pool = ctx.enter_context(tc.tile_pool(name="cf", bufs=4))
        wrk = ctx.enter_context(tc.tile_pool(name="wrk", bufs=1))
        ypool = ctx.enter_context(tc.tile_pool(name="yp", bufs=2))
        psB = ctx.enter_context(tc.tile_pool(name="psB", bufs=4, space="PSUM"))

        def build_coef(ring, dir_, k):
            """Broadcast coef rows for chunk k to a [128, 2*CH] tile
            ([dn | dp]) via a depth-4 doubling tree off 8-replica rows."""
            ct = cf_pool.tile([128, 2 * CH], F16, tag="ct",
                              name=f"ct{dir_}_{k}")
            i0 = k * CH
            hh, off = i0 // (H // 2), i0 % (H // 2)
            for h in (0, 1):
                r0 = dir_ * 64 + (h * 2 + hh) * 16
                ring.dma_start(ct[h * 64:h * 64 + 8, :],
                               flat[r0:r0 + 16, off:off + CH])
            for p in (8, 16, 32):
                for b in (0, 64):
                    ring.dma_start(ct[b + p:b + 2 * p, :], ct[b:b + p, :])
            return ct

        def warp(src, s_off, ct, dst, d_off, n):
            """dst = src + dn*(src[-1]-src) + dp*(src[+1]-src) over n cols."""
            s1 = wrk.tile([128, CH], F16, tag="s1")
            s2 = wrk.tile([128, CH], F16, tag="s2")
            nc.vector.tensor_tensor(s1[:, 0:n], src[:, s_off - 1:s_off - 1 + n],
                                    src[:, s_off:s_off + n], ALU.subtract)
            nc.vector.tensor_tensor(s1[:, 0:n], s1[:, 0:n], ct[:, 0:n], ALU.mult)
            nc.vector.tensor_tensor(s2[:, 0:n], src[:, s_off + 1:s_off + 1 + n],
                                    src[:, s_off:s_off + n], ALU.subtract)
            nc.vector.tensor_tensor(s2[:, 0:n], s2[:, 0:n],
                                    ct[:, CH:CH + n], ALU.mult)
            nc.vector.tensor_tensor(dst[:, d_off:d_off + n],
                                    src[:, s_off:s_off + n], s1[:, 0:n], ALU.add)
            nc.vector.tensor_tensor(dst[:, d_off:d_off + n],
                                    dst[:, d_off:d_off + n], s2[:, 0:n], ALU.add)

        # fwd builds split across sync/scalar rings, inv on the gpsimd ring
        flatten(0, nc.sync)
        cts = {}
        cts[3] = build_coef(nc.sync, 0, 3)
        cts[0] = build_coef(nc.scalar, 0, 0)
        cts[1] = build_coef(nc.sync, 0, 1)
        cts[2] = build_coef(nc.scalar, 0, 2)
        flatten(1, nc.gpsimd)
        cis = {k: build_coef(nc.gpsimd, 1, k) for k in range(NCH)}

        # forward warps, ordered so the cross-half conv seam is ready early
        for k in (3, 0, 1, 2):
            i0 = k * CH
            warp(x16, 1 + i0, cts[k], xw, 1 + i0, CH)
            if k == 3:   # half1's left conv halo = last warped col of half0
                nc.sync.dma_start(xw[64:128, 0:1], xw[0:64, H:H + 1])
            if k == 0:   # half0's right conv halo = first warped col of half1
                nc.sync.dma_start(xw[0:64, H + 1:H + 2], xw[64:128, 1:2])

        # conv + inverse warp + store, per chunk
        for k in range(NCH):
            i0 = k * CH
            y16 = ypool.tile([128, CH + 2], F16, tag="y16")
            nc.vector.memset(y16[:, 0:1], 0.0)
            nc.vector.memset(y16[:, CH + 1:CH + 2], 0.0)
            for b in range(CH // 512):
                pc = psB.tile([128, 512], F32, tag="pc")
                for j in range(3):
                    nc.tensor.matmul(pc[:], cw_sb[:, j * 128:(j + 1) * 128],
                                     xw[:, i0 + b * 512 + j:i0 + b * 512 + j + 512],
                                     start=(j == 0), stop=(j == 2))
                nc.scalar.activation(y16[:, 1 + b * 512:1 + (b + 1) * 512], pc[:],
                                     AF.Identity, bias=cb_sb[:])
            warp(y16, 1, cis[k], y16, 1, CH)   # in-place: out = y + s1 + s2
            for h in (0, 1):
                nc.sync.dma_start(y_h[h][:, i0:i0 + CH],
                                  y16[h * 64:(h + 1) * 64, 1:1 + CH])

    nc.compile()
    return nc


def _host_params(est_w, conv_w, conv_b):
    ew = np.zeros((128, 2), np.float32)
    ew[:64, 0] = est_w
    ew[64:, 1] = est_w
    cw = np.zeros((128, 384), np.float16)
    for j in range(3):
        blk = conv_w[:, :, j].T.astype(np.float16)   # (in, out)
        cw[0:64, j * 128:j * 128 + 64] = blk
        cw[64:128, j * 128 + 64:j * 128 + 128] = blk
    cb = np.concatenate([conv_b, conv_b]).astype(np.float32)[:, None]
    return ew, cw, cb


_COMPILED = None


def _get_compiled():
    global _COMPILED
    if _COMPILED is None:
        nc = _build_module()
        nc.m = get_hw_module(nc.m)
        _COMPILED = nc
    return _COMPILED


def kernel(signal, est_w, conv_w, conv_b, _trace=False, _trace_kwargs=None):
    nc = _get_compiled()
    ew, cw, cb = _host_params(np.asarray(est_w, np.float32),
                              np.asarray(conv_w, np.float32),
                              np.asarray(conv_b, np.float32))
    signal = np.ascontiguousarray(np.asarray(signal, np.float32))
    in_maps = [{"x": signal[b], "ew": ew, "cw": cw, "cb": cb}
               for b in range(NCORES)]
    res = bass_utils.run_bass_kernel_spmd(
        nc, in_maps, core_ids=list(range(NCORES)), trace=_trace,
        **(_trace_kwargs or {}))
    out = np.stack([np.asarray(r["y"], np.float32) for r in res.results], axis=0)
    if _trace:
        return out, res
    return out


# revision 42
# speedup vs baseline: 1.1712x; 1.1712x over previous
"""Trainium2 Bass kernel for gated short-time-warp + Conv1d (nn_GW_Conv1D).

Reference computation (per batch element b, C=64 channels, T=32768):
  g = tanh(einsum('ct,c->t', x, est_w)) * 0.5            # velocity, |g| <= 0.5
  d = flow(g)    per 256-window (scaling & squaring, 4 iters), |d| <= 0.5
  xw = interp1d(x, p + d)   per window                    # forward warp
  y = conv1d(xw, conv_w, conv_b, k=3, SAME)               # channel mixing
  d_inv = flow(-g); out = interp1d(y, p + d_inv)          # inverse warp

|d| < 1 always, so each warp is a 3-term elementwise stencil:
  out = x*(1-dn-dp) + x[-1]*dn + x[+1]*dp,  dn=relu(-d), dp=relu(d)
with dn zeroed at window starts and dp zeroed at window ends (the clip).
The edge zeroing is folded into d itself before broadcasting:
  d[win col 0] <- max(d, 0)   (kills dn only; dp unchanged)
  d[win col W-1] <- min(d, 0) (kills dp only; dn unchanged)

v10 architecture (single streaming pass, fp16 datapath):
  - "halves" layout everywhere: partition p = (h, c), free dim = t in half.
  - x streamed fp32 in two phases through the xw tile's memory (bitcast
    staging), h0 on the sync ring / h1 on the scalar ring (disjoint
    even/odd SDMA engine sets); the fp32 einsum runs straight off the
    staging while the DVE casts it to the persistent fp16 x tile.
  - flow on [128 windows, 512] fp16 (fwd | inv), all-DVE.
  - per-window warp coefficients: derive dn/dp small, flatten to 8-replica
    (dn,dp)-interleaved rows, then per chunk an 8-DMA depth-4 doubling
    tree broadcasts to [128, 2*CH] (fwd split over sync+scalar rings,
    inv on the gpsimd SWDGE ring; stores on sync).
  - 6-op warp entirely on DVE fp16 2x: s1=(x[-1]-x)*dn; s2=(x[+1]-x)*dp;
    out = x+s1+s2.
  - conv as block-diagonal K=128 fp16 matmuls, bias fused in the
    scalar-engine PSUM evacuation.
  - output stored fp16 (host casts to fp32).

Sharding: pure data parallelism, batch b -> core b (8 cores).
"""
import sys

sys.path.insert(0, "/opt/trn_rl_repo")

import numpy as np
from contextlib import ExitStack

import concourse.bass as bass
import concourse.tile as tile
from concourse import bacc, mybir
from concourse.bass_interp import get_hw_module
from concourse import bass_utils

F32 = mybir.dt.float32
F16 = mybir.dt.float16
AF = mybir.ActivationFunctionType
ALU = mybir.AluOpType

NCORES = 8
C, T, W = 64, 32768, 256
H = T // 2            # 16384 cols per half
FLOW_ITERS = 4
CH = 4096             # main-loop chunk (window-aligned)
NCH = H // CH         # 4 chunks


def _flow_dir(nc, pool, d, sfx):
    """Scaling-and-squaring on one direction d (128, 256) fp16, all-DVE."""
    for _ in range(FLOW_ITERS):
        dn = pool.tile([128, 256], F16, tag="fl_dn" + sfx)
        dp = pool.tile([128, 256], F16, tag="fl_dp" + sfx)
        nc.vector.tensor_scalar(dn[:], d[:], -1.0, 0.0, ALU.mult, ALU.max)
        nc.vector.tensor_scalar_max(dp[:], d[:], 0.0)
        nc.vector.tensor_scalar_mul(dn[:, 0:1], dn[:, 0:1], 0.0)
        nc.vector.tensor_scalar_mul(dp[:, 255:256], dp[:, 255:256], 0.0)
        am = pool.tile([128, 256], F16, tag="fl_am" + sfx)
        nc.vector.tensor_tensor(am[:], dn[:], dp[:], ALU.add)
        nc.vector.tensor_scalar(am[:], am[:], -1.0, 1.0, ALU.mult, ALU.add)
        itp = pool.tile([128, 256], F16, tag="fl_it" + sfx)
        tmp = pool.tile([128, 256], F16, tag="fl_tm" + sfx)
        nc.vector.tensor_tensor(itp[:], d[:], am[:], ALU.mult)
        nc.vector.tensor_tensor(tmp[:, 1:256], d[:, 0:255], dn[:, 1:256], ALU.mult)
        nc.vector.tensor_tensor(itp[:, 1:256], itp[:, 1:256], tmp[:, 1:256], ALU.add)
        nc.vector.tensor_tensor(tmp[:, 0:255], d[:, 1:256], dp[:, 0:255], ALU.mult)
        nc.vector.tensor_tensor(itp[:, 0:255], itp[:, 0:255], tmp[:, 0:255], ALU.add)
        nc.vector.tensor_tensor(d[:], d[:], itp[:], ALU.add)
    # fold the window-edge clip into d itself
    nc.vector.tensor_scalar_max(d[:, 0:1], d[:, 0:1], 0.0)
    nc.vector.tensor_scalar_min(d[:, 255:256], d[:, 255:256], 0.0)


def _build_module():
    nc = bacc.Bacc("TRN2", target_bir_lowering=False, debug=False,
                   enable_asserts=False, num_devices=NCORES)
    x = nc.dram_tensor("x", (C, T), F32, kind="ExternalInput").ap()
    ew = nc.dram_tensor("ew", (128, 2), F16, kind="ExternalInput").ap()
    cw = nc.dram_tensor("cw", (128, 384), F16, kind="ExternalInput").ap()
    cb = nc.dram_tensor("cb", (128, 1), F32, kind="ExternalInput").ap()
    y = nc.dram_tensor("y", (C, T), F16, kind="ExternalOutput").ap()

    # per-half views: [64, H] slices of the (C, T) tensors (outer dim 64
    # keeps DMA descriptors striped across many SDMA engines)
    x_h = [x[:, 0:H], x[:, H:T]]
    y_h = [y[:, 0:H], y[:, H:T]]

    with tile.TileContext(nc) as tc, ExitStack() as ctx:
        big = ctx.enter_context(tc.tile_pool(name="big", bufs=1))
        sm = ctx.enter_context(tc.tile_pool(name="sm", bufs=1))

        # persistent tiles: x (halo'd), warped x (halo'd), flat coef rows
        x16 = big.tile([128, H + 2], F16)
        xw = big.tile([128, H + 2], F16)
        # flat coef rows: r = dir*64 + (h*2+hh)*16 + rep*2 + ci
        # (8 replicated (dn,dp) row pairs per (dir,h,hh)), cols = H/2 span hh
        flat = big.tile([128, H // 2], F16)

        ew_sb = sm.tile([128, 2], F16, tag="ew")
        nc.sync.dma_start(ew_sb[:], ew)
        cw_sb = sm.tile([128, 384], F16, tag="cw")
        nc.sync.dma_start(cw_sb[:], cw)
        cb_sb = sm.tile([128, 1], F32, tag="cb")
        nc.sync.dma_start(cb_sb[:], cb)

        # ------- Stage A: cast-load x, fp16 einsum chasing ------------------
        nc.gpsimd.memset(x16[:, 0:1], 0.0)
        nc.gpsimd.memset(x16[:, H + 1:H + 2], 0.0)
        nc.gpsimd.memset(xw[:, 0:1], 0.0)
        nc.gpsimd.memset(xw[:, H + 1:H + 2], 0.0)
        # h0/h1 DMA pairs hit disjoint (even/odd) SDMA engine sets
        for i in (0, H // 2):
            for h in (0, 1):
                nc.gpsimd.dma_start(x16[h * 64:(h + 1) * 64, 1 + i:1 + i + H // 2],
                                    x_h[h][:, i:i + H // 2])
        g_w = sm.tile([128, 256], F16, tag="gw")         # windows on partitions
        stageA = ctx.enter_context(tc.tile_pool(name="stA", bufs=2))
        with tc.tile_pool(name="psA", bufs=2, space="PSUM") as psA:
            for i in range(0, H, 2048):
                pg = psA.tile([2, 2048], F32, tag="pg")
                for j in range(4):
                    nc.tensor.matmul(pg[:, j * 512:(j + 1) * 512], ew_sb[:],
                                     x16[:, 1 + i + j * 512:1 + i + (j + 1) * 512],
                                     start=True, stop=True)
                ge = stageA.tile([2, 2048], F16, tag="ge")
                nc.scalar.copy(ge[:], pg[:])
                w0 = i // 256
                for h in (0, 1):
                    nc.sync.dma_start(g_w[h * 64 + w0:h * 64 + w0 + 8, :],
                                      ge[h:h + 1, :])

        # ------- Stage B: tanh, per-direction flow/derive/flatten -----------
        g_th = sm.tile([128, 256], F32, tag="gth")
        nc.scalar.activation(g_th[:], g_w[:], AF.Tanh)

        def derive_flatten(dir_, ring):
            d = sm.tile([128, 256], F16, tag=f"d{dir_}", name=f"d{dir_}")
            nc.vector.tensor_scalar_mul(d[:], g_th[:],
                                        0.5 / 16.0 if dir_ == 0 else -0.5 / 16.0)
            _flow_dir(nc, sm, d, str(dir_))
            dn_s = sm.tile([128, 256], F16, tag=f"dn{dir_}", name=f"dn{dir_}")
            dp_s = sm.tile([128, 256], F16, tag=f"dp{dir_}", name=f"dp{dir_}")
            nc.vector.tensor_scalar(dn_s[:], d[:], -1.0, 0.0, ALU.mult, ALU.max)
            nc.vector.tensor_scalar_max(dp_s[:], d[:], 0.0)
            for ci_, coef_s in ((0, dn_s), (1, dp_s)):
                for h in (0, 1):
                    for hh in (0, 1):
                        r = dir_ * 64 + (h * 2 + hh) * 16 + ci_
                        ring.dma_start(
                            flat[r:r + 1, :],
                            coef_s[h * 64 + hh * 32:h * 64 + (hh + 1) * 32, :])
            # replicate each (dn,dp) row pair to 8 copies, wave-major
            for p in (2, 4, 8):
                for blk in range(4):
                    r0 = dir_ * 64 + blk * 16
                    ring.dma_start(flat[r0 + p:r0 + 2 * p, :],
                                   flat[r0:r0 + p, :])

        # ------- main streaming loop ----------------------------------------
        cf_pool = ctx.enter_context(tc.tile_pool(name="cf", bufs=4))
        wrk = ctx.enter_context(tc.tile_pool(name="wrk", bufs=1))
        ypool = ctx.enter_context(tc.tile_pool(name="yp", bufs=2))
        psB = ctx.enter_context(tc.tile_pool(name="psB", bufs=4, space="PSUM"))

        def build_coef(ring, dir_, k):
            """Broadcast coef rows for chunk k to a [128, 2*CH] tile
            ([dn | dp]) via a depth-4 doubling tree off 8-replica rows."""
            ct = cf_pool.tile([128, 2 * CH], F16, tag="ct",
                              name=f"ct{dir_}_{k}")
            i0 = k * CH
            hh, off = i0 // (H // 2), i0 % (H // 2)
            for h in (0, 1):
                r0 = dir_ * 64 + (h * 2 + hh) * 16
                ring.dma_start(ct[h * 64:h * 64 + 8, :],
                               flat[r0:r0 + 16, off:off + CH])
            for p in (8, 16, 32):
                for b in (0, 64):
                    ring.dma_start(ct[b + p:b + 2 * p, :], ct[b:b + p, :])
            return ct

        def warp(src, s_off, ct, dst, d_off, n):
            """dst = src + dn*(src[-1]-src) + dp*(src[+1]-src) over n cols."""
            s1 = wrk.tile([128, CH], F16, tag="s1")
            s2 = wrk.tile([128, CH], F16, tag="s2")
            nc.vector.tensor_tensor(s1[:, 0:n], src[:, s_off - 1:s_off - 1 + n],
                                    src[:, s_off:s_off + n], ALU.subtract)
            nc.vector.tensor_tensor(s1[:, 0:n], s1[:, 0:n], ct[:, 0:n], ALU.mult)
            nc.vector.tensor_tensor(s2[:, 0:n], src[:, s_off + 1:s_off + 1 + n],
                                    src[:, s_off:s_off + n], ALU.subtract)
            nc.vector.tensor_tensor(s2[:, 0:n], s2[:, 0:n],
                                    ct[:, CH:CH + n], ALU.mult)
            nc.vector.tensor_tensor(dst[:, d_off:d_off + n],
                                    src[:, s_off:s_off + n], s1[:, 0:n], ALU.add)
            nc.vector.tensor_tensor(dst[:, d_off:d_off + n],
                                    dst[:, d_off:d_off + n], s2[:, 0:n], ALU.add)

        # fwd builds split across sync/scalar rings, inv on the gpsimd ring
        derive_flatten(0, nc.sync)
        cts = {}
        cts[3] = build_coef(nc.sync, 0, 3)
        cts[0] = build_coef(nc.scalar, 0, 0)
        cts[1] = build_coef(nc.sync, 0, 1)
        cts[2] = build_coef(nc.scalar, 0, 2)
        derive_flatten(1, nc.gpsimd)
        cis = {k: build_coef(nc.gpsimd, 1, k) for k in range(NCH)}

        # forward warps, ordered so the cross-half conv seam is ready early
        for k in (3, 0, 1, 2):
            i0 = k * CH
            warp(x16, 1 + i0, cts[k], xw, 1 + i0, CH)
            if k == 3:   # half1's left conv halo = last warped col of half0
                nc.sync.dma_start(xw[64:128, 0:1], xw[0:64, H:H + 1])
            if k == 0:   # half0's right conv halo = first warped col of half1
                nc.sync.dma_start(xw[0:64, H + 1:H + 2], xw[64:128, 1:2])

        # conv + inverse warp + store, per chunk
        for k in range(NCH):
            i0 = k * CH
            y16 = ypool.tile([128, CH + 2], F16, tag="y16")
            nc.vector.memset(y16[:, 0:1], 0.0)
            nc.vector.memset(y16[:, CH + 1:CH + 2], 0.0)
            for b in range(CH // 512):
                pc = psB.tile([128, 512], F32, tag="pc")
                for j in range(3):
                    nc.tensor.matmul(pc[:], cw_sb[:, j * 128:(j + 1) * 128],
                                     xw[:, i0 + b * 512 + j:i0 + b * 512 + j + 512],
                                     start=(j == 0), stop=(j == 2))
                nc.scalar.activation(y16[:, 1 + b * 512:1 + (b + 1) * 512], pc[:],
                                     AF.Identity, bias=cb_sb[:])
            warp(y16, 1, cis[k], y16, 1, CH)   # in-place: out = y + s1 + s2
            for h in (0, 1):
                nc.sync.dma_start(y_h[h][:, i0:i0 + CH],
                                  y16[h * 64:(h + 1) * 64, 1:1 + CH])

    nc.compile()
    return nc


def _host_params(est_w, conv_w, conv_b):
    ew = np.zeros((128, 2), np.float16)
    ew[:64, 0] = est_w
    ew[64:, 1] = est_w
    cw = np.zeros((128, 384), np.float16)
    for j in range(3):
        blk = conv_w[:, :, j].T.astype(np.float16)   # (in, out)
        cw[0:64, j * 128:j * 128 + 64] = blk
        cw[64:128, j * 128 + 64:j * 128 + 128] = blk
    cb = np.concatenate([conv_b, conv_b]).astype(np.float32)[:, None]
    return ew, cw, cb


_COMPILED = None


def _get_compiled():
    global _COMPILED
    if _COMPILED is None:
        nc = _build_module()
        nc.m = get_hw_module(nc.m)
        _COMPILED = nc
    return _COMPILED


def kernel(signal, est_w, conv_w, conv_b, _trace=False, _trace_kwargs=None):
    nc = _get_compiled()
    ew, cw, cb = _host_params(np.asarray(est_w, np.float32),
                              np.asarray(conv_w, np.float32),
                              np.asarray(conv_b, np.float32))
    signal = np.ascontiguousarray(np.asarray(signal, np.float32))
    in_maps = [{"x": signal[b], "ew": ew, "cw": cw, "cb": cb}
               for b in range(NCORES)]
    res = bass_utils.run_bass_kernel_spmd(
        nc, in_maps, core_ids=list(range(NCORES)), trace=_trace,
        **(_trace_kwargs or {}))
    out = np.stack([np.asarray(r["y"], np.float32) for r in res.results], axis=0)
    if _trace:
        return out, res
    return out
